# revision 15
# baseline (speedup 1.0000x reference)
"""NonLocalAttention (embedded gaussian, no softmax) on 8 trn2 NeuronCores.

Reference math (per sample, all linear — no softmax):
    theta = conv1x1(a, theta_w, theta_b)        # [Ci, N]
    phi   = conv1x1(b, phi_w, phi_b)            # [Ci, N]
    g     = conv1x1(b, g_w, g_b)                # [Ci, N]
    f     = theta^T @ phi / N                   # [N, N]
    y     = f @ g^T                             # [N, Ci]
    out   = BN(W_w @ y^T)                       # [C, N]

Associativity rewrite: there is no nonlinearity between the two big
matmuls, so the NxN attention map need never be materialized:
    Mi[ci1, ci2] = sum_m phi[ci1, m] * g[ci2, m]          # [128, 128]
    y^T[ci2, n]  = sum_ci1 Mi[ci1, ci2] * theta[ci1, n]   # (1/N in theta)
Per-core compute drops from ~2.1 GMAC to ~0.07 GMAC; the kernel runs near
the memory roofline.

Sharding: 8 cores = 4 samples x 2 pixel-halves. Core (s, h) loads only its
half of a AND b: it computes the theta conv on its half of a, the phi/g
convs + a partial Mi over its half of b's pixels, then the core pair
AllReduce-adds the [128,128] partial Mi (the Mi contraction is a plain sum
over pixels, and the per-channel conv biases are applied inside each half
before the product, so partial sums add exactly). The theta conv is
scheduled during the collective's latency window. No other communication.

The Mi contraction runs over pixels, which needs phi/g tiles with pixels on
partitions: produced by PE transpose-mode (matmul vs identity) on 128x128
tiles of the conv outputs. Transpose evictions alternate DVE / ACT engines.

DMAs are batched coarsely (each dma_start costs ~0.6us of serial issue) and
ordered consts -> b -> a so phi/g convs start as early as possible.

PRECISION = "f32" (default): all matmuls in true fp32 (PE LOW+HIGH
two-pass mode); output matches the jax reference to ~7e-7 relative.
PRECISION = "f32r": TF32-like single-pass PE mode, 4x faster matmuls,
~3e-4 relative error end-to-end (measured on HW).
"""

import numpy as np

B, C, Ci, H, W = 4, 256, 128, 64, 64
N_PIX = H * W            # 4096 pixels per sample
N_CORES = 8
HALF = N_PIX // 2        # 2048 pixels per core (output AND b-contraction)
P = 128
CC = C // P              # 2 channel chunks
RB = 512                 # row block (max 4-byte moving free dim)
MCH_H = HALF // P        # 16 pixel chunks for the partial-Mi contraction
BN_EPS = 1e-5

PRECISION = "f32"        # "f32r" | "f32"

_CACHE = {}


def _build(precision=PRECISION):
    import concourse.bacc as bacc
    import concourse.mybir as mybir
    import concourse.tile as tile
    from concourse.masks import make_identity

    f32 = mybir.dt.float32
    fmm = mybir.dt.float32r if precision == "f32r" else f32
    Act = mybir.ActivationFunctionType

    # Bacc (not raw Bass): compile() legalizes sync waits (TRN2 allows at
    # most one sem wait per instruction; excess waits split onto
    # InstEventSemaphore / moved to ldweights).
    nc = bacc.Bacc("TRN2", num_devices=N_CORES)

    # packed weights: [thetaT(2x128) | phiT(2x128) | gwT(2x128) | WT(256)]
    wpack_d = nc.dram_tensor("wpack", [P, 4 * C], fmm, kind="ExternalInput")
    # packed f32 per-partition vectors:
    # [tb | pb | gb | scale cc0 | scale cc1 | shift cc0 | shift cc1]
    vpack_d = nc.dram_tensor("vpack", [P, 7], f32, kind="ExternalInput")
    a_d = nc.dram_tensor("a_half", [CC, P, HALF], fmm, kind="ExternalInput")
    b_d = nc.dram_tensor("b_half", [CC, P, HALF], fmm, kind="ExternalInput")
    out_d = nc.dram_tensor("out", [CC, P, HALF], f32, kind="ExternalOutput")
    # pairwise Mi AllReduce bounce buffers (plain Local DRAM — the 2-core
    # collective path rejects Shared scratchpad on either side)
    mi_in_d = nc.dram_tensor("mi_cc_in", [Ci, Ci], f32)
    mi_out_d = nc.dram_tensor("mi_cc_out", [Ci, Ci], f32)
    cc_groups = [[2 * s, 2 * s + 1] for s in range(4)]

    with tile.TileContext(nc) as tc:
        with (
            tc.tile_pool(name="const", bufs=1) as cpool,
            tc.tile_pool(name="big", bufs=1) as bpool,
            tc.tile_pool(name="work", bufs=3) as wpool,
            tc.tile_pool(name="ps", bufs=2, space="PSUM") as ppool,
        ):
            # ---- constants: 2 DMAs -----------------------------------------
            wpack_sb = cpool.tile([P, 4 * C], fmm)
            vpack_sb = cpool.tile([P, 7], f32)
            nc.sync.dma_start(out=wpack_sb[:], in_=wpack_d[:])
            nc.sync.dma_start(out=vpack_sb[:], in_=vpack_d[:])
            thetaT_sb = wpack_sb[:, 0:C].rearrange("p (c k) -> p c k", c=CC)
            phiT_sb = wpack_sb[:, C : 2 * C].rearrange("p (c k) -> p c k", c=CC)
            gwT_sb = wpack_sb[:, 2 * C : 3 * C].rearrange("p (c k) -> p c k", c=CC)
            WT_sb = wpack_sb[:, 3 * C : 4 * C]
            tb_sb, pb_sb, gb_sb = (vpack_sb[:, i : i + 1] for i in range(3))
            scale_sb = vpack_sb[:, 3:5]
            shift_sb = vpack_sb[:, 5:7]

            # gpsimd memset/affine_select reject f32r and the BIR verifier
            # wants f32r matmul operands produced as f32r: build the identity
            # in f32, then round it into the matmul dtype with a DVE copy
            ident_f32 = cpool.tile([P, P], f32)
            ident_sb = cpool.tile([P, P], fmm)
            make_identity(nc, ident_f32[:])
            nc.vector.tensor_copy(ident_sb[:], ident_f32[:])

            # ---- activation loads: b first (phi/g + Mi), then a ------------
            a_sb = bpool.tile([P, CC, HALF], fmm)
            b_sb = bpool.tile([P, CC, HALF], fmm)
            for hh in range(2):
                s = hh * (HALF // 2)
                for cc in range(CC):
                    nc.sync.dma_start(
                        out=b_sb[:, cc, s : s + HALF // 2],
                        in_=b_d[cc, :, s : s + HALF // 2],
                    )
            for cc in range(CC):
                nc.sync.dma_start(out=a_sb[:, cc, :], in_=a_d[cc])

            # ---- phi/g convs + transpose + partial Mi, per 1024-px half ----
            phi_x = bpool.tile([Ci, HALF], fmm)
            g_x = bpool.tile([Ci, HALF], fmm)
            mi_ps = ppool.tile([Ci, Ci], f32, tag="mi", bufs=1, name="mi_ps")
            for q in range(2):
                for p in (2 * q, 2 * q + 1):
                    sl = slice(p * RB, (p + 1) * RB)
                    ph_ps = ppool.tile([P, RB], f32, tag="conv", bufs=2, name="ph_ps")
                    for cc in range(CC):
                        nc.tensor.matmul(
                            ph_ps[:],
                            phiT_sb[:, cc, :],
                            b_sb[:, cc, sl],
                            start=(cc == 0),
                            stop=(cc == CC - 1),
                        )
                    nc.scalar.activation(
                        phi_x[:, sl], ph_ps[:], Act.Identity, bias=pb_sb,
                    )
                    g_ps = ppool.tile([P, RB], f32, tag="conv", bufs=2, name="g_ps")
                    for cc in range(CC):
                        nc.tensor.matmul(
                            g_ps[:],
                            gwT_sb[:, cc, :],
                            b_sb[:, cc, sl],
                            start=(cc == 0),
                            stop=(cc == CC - 1),
                        )
                    nc.scalar.activation(
                        g_x[:, sl], g_ps[:], Act.Identity, bias=gb_sb,
                    )
                for m in range(8 * q, 8 * q + 8):
                    sl = slice(m * P, (m + 1) * P)
                    tpp_ps = ppool.tile([P, P], fmm, tag="tp", bufs=3, name="tpp_ps")
                    nc.tensor.transpose(tpp_ps[:], phi_x[:, sl], ident_sb[:])
                    phiT_m = wpool.tile([P, Ci], fmm, tag="phiT_m", bufs=3,
                                        name="phiT_m")
                    nc.vector.tensor_copy(phiT_m[:], tpp_ps[:])
                    tpg_ps = ppool.tile([P, P], fmm, tag="tp", bufs=3, name="tpg_ps")
                    nc.tensor.transpose(tpg_ps[:], g_x[:, sl], ident_sb[:])
                    gT_m = wpool.tile([P, Ci], fmm, tag="gT_m", bufs=3, name="gT_m")
                    nc.scalar.activation(gT_m[:], tpg_ps[:], Act.Copy)
                    nc.tensor.matmul(
                        mi_ps[:], phiT_m[:], gT_m[:],
                        start=(m == 0), stop=(m == MCH_H - 1),
                    )

            # ---- pairwise AllReduce of the partial Mi ----------------------
            mi_part_sb = wpool.tile([Ci, Ci], f32, tag="mi_p", bufs=1,
                                    name="mi_part_sb")
            nc.vector.tensor_copy(mi_part_sb[:], mi_ps[:])
            nc.sync.dma_start(out=mi_in_d[:], in_=mi_part_sb[:])
            nc.gpsimd.collective_compute(
                "AllReduce", mybir.AluOpType.add,
                ins=[mi_in_d[:]], outs=[mi_out_d[:]], replica_groups=cc_groups,
            )
            mi_f32 = wpool.tile([Ci, Ci], f32, tag="mi_f", bufs=1, name="mi_f32")
            nc.sync.dma_start(out=mi_f32[:], in_=mi_out_d[:])
            if fmm is f32:
                mi_sb = mi_f32
            else:
                mi_sb = wpool.tile([Ci, Ci], fmm, tag="mi_sb", bufs=1, name="mi_sb")
                nc.vector.tensor_copy(mi_sb[:], mi_f32[:])

            # ---- theta conv (fills the collective latency window) ----------
            theta_x = bpool.tile([Ci, HALF], fmm)
            for p in range(HALF // RB):
                th_ps = ppool.tile([P, RB], f32, tag="conv", bufs=2, name="th_ps")
                for cc in range(CC):
                    nc.tensor.matmul(
                        th_ps[:],
                        thetaT_sb[:, cc, :],
                        a_sb[:, cc, p * RB : (p + 1) * RB],
                        start=(cc == 0),
                        stop=(cc == CC - 1),
                    )
                nc.scalar.activation(
                    theta_x[:, p * RB : (p + 1) * RB], th_ps[:], Act.Identity,
                    bias=tb_sb,
                )

            # ---- y^T = Mi^T-contract theta_x; W conv; BN; store ------------
            for r in range(HALF // RB):
                rows = slice(r * RB, (r + 1) * RB)
                yt_ps = ppool.tile([Ci, RB], f32, tag="yt", bufs=2, name="yt_ps")
                nc.tensor.matmul(
                    yt_ps[:], mi_sb[:], theta_x[:, rows], start=True, stop=True,
                )
                yT_sb = wpool.tile([Ci, RB], fmm, tag="ysb", bufs=2, name="yT_sb")
                nc.vector.tensor_copy(yT_sb[:], yt_ps[:])
                osb = wpool.tile([P, CC, RB], f32, tag="osb", bufs=2, name="osb")
                for cc in range(CC):
                    wy_ps = ppool.tile([P, RB], f32, tag="conv", bufs=2, name="wy_ps")
                    nc.tensor.matmul(
                        wy_ps[:],
                        WT_sb[:, cc * P : (cc + 1) * P],
                        yT_sb[:],
                        start=True,
                        stop=True,
                    )
                    nc.scalar.activation(
                        osb[:, cc, :], wy_ps[:], Act.Identity,
                        bias=shift_sb[:, cc : cc + 1],
                        scale=scale_sb[:, cc : cc + 1],
                    )
                nc.sync.dma_start(
                    out=out_d[:, :, rows].rearrange("c p r -> p c r"), in_=osb[:],
                )

    nc.compile()
    return nc


def _get_nc():
    if "nc" not in _CACHE:
        _CACHE["nc"] = _build()
    return _CACHE["nc"]


def _prep_in_maps(a, b, theta_w, theta_b, phi_w, phi_b, g_w, g_b, W_w,
                  bn_gamma, bn_beta, bn_mean, bn_var):
    f = np.float32
    a4 = np.ascontiguousarray(np.asarray(a, f).reshape(B, C, N_PIX))
    b4 = np.ascontiguousarray(np.asarray(b, f).reshape(B, C, N_PIX))

    inv_n = 1.0 / np.float64(N_PIX)
    thetaT = (np.asarray(theta_w, f).T * inv_n).astype(f)   # [C, Ci]
    phiT = np.asarray(phi_w, f).T                           # [C, Ci]
    gwT = np.asarray(g_w, f).T                              # [C, Ci]
    WT = np.asarray(W_w, f).T                               # [Ci, C]
    # wpack rows: partition p; cols: [thetaT cc0|cc1 | phiT cc0|cc1 |
    #                                 gwT cc0|cc1 | WT]
    wpack = np.empty((P, 4 * C), f)
    for i, wT in enumerate((thetaT, phiT, gwT)):
        for cc in range(CC):
            wpack[:, i * C + cc * Ci : i * C + (cc + 1) * Ci] = \
                wT[cc * P : (cc + 1) * P, :]
    wpack[:, 3 * C : 4 * C] = WT
    wpack = np.ascontiguousarray(wpack)

    scale = (np.asarray(bn_gamma, f) / np.sqrt(np.asarray(bn_var, f) + BN_EPS)).astype(f)
    shift = (np.asarray(bn_beta, f) - np.asarray(bn_mean, f) * scale).astype(f)
    vpack = np.stack([
        (np.asarray(theta_b, f) * inv_n).astype(f),
        np.asarray(phi_b, f),
        np.asarray(g_b, f),
        scale[:P], scale[P:],
        shift[:P], shift[P:],
    ], axis=1)
    vpack = np.ascontiguousarray(vpack)

    in_maps = []
    for core in range(N_CORES):
        s, h = divmod(core, 2)
        sl = slice(h * HALF, (h + 1) * HALF)
        in_maps.append({
            "a_half": np.ascontiguousarray(a4[s][:, sl].reshape(CC, P, HALF)),
            "b_half": np.ascontiguousarray(b4[s][:, sl].reshape(CC, P, HALF)),
            "wpack": wpack,
            "vpack": vpack,
        })
    return in_maps


def run(inputs: dict, trace: bool = False):
    from concourse.bass_utils import run_bass_kernel_spmd

    nc = _get_nc()
    in_maps = _prep_in_maps(**inputs)
    res = run_bass_kernel_spmd(nc, in_maps, list(range(N_CORES)), trace=trace)
    out = np.empty((B, C, N_PIX), np.float32)
    for core in range(N_CORES):
        s, h = divmod(core, 2)
        out[s][:, h * HALF : (h + 1) * HALF] = res.results[core]["out"].reshape(C, HALF)
    return out.reshape(B, C, H, W), res


def kernel(**inputs) -> np.ndarray:
    out, _ = run(inputs, trace=False)
    return out


# revision 16
# speedup vs baseline: 1.1752x; 1.1752x over previous
"""NonLocalAttention (embedded gaussian, no softmax) on 8 trn2 NeuronCores.

Reference math (per sample, all linear — no softmax):
    theta = conv1x1(a, theta_w, theta_b)        # [Ci, N]
    phi   = conv1x1(b, phi_w, phi_b)            # [Ci, N]
    g     = conv1x1(b, g_w, g_b)                # [Ci, N]
    f     = theta^T @ phi / N                   # [N, N]
    y     = f @ g^T                             # [N, Ci]
    out   = BN(W_w @ y^T)                       # [C, N]

Associativity rewrite: there is no nonlinearity between the two big
matmuls, so the NxN attention map need never be materialized:
    Mi[ci1, ci2] = sum_m phi[ci1, m] * g[ci2, m]          # [128, 128]
    y^T[ci2, n]  = sum_ci1 Mi[ci1, ci2] * theta[ci1, n]   # (1/N in theta)
Per-core compute drops from ~2.1 GMAC to ~0.13 GMAC; the kernel runs near
the memory roofline.

Sharding: 8 cores = 4 samples x 2 pixel-halves. Core (s, h) computes output
pixels [h*2048, (h+1)*2048) of sample s: theta conv on its half of a, phi/g
convs + Mi on the full b (duplicated across the 2 cores of a sample, cheap),
zero inter-core communication.

The Mi contraction runs over pixels, which needs phi/g tiles with pixels on
partitions: produced by PE transpose-mode (matmul vs identity) on 128x128
tiles of the conv outputs. Transpose evictions alternate DVE / ACT engines.

DMAs are batched coarsely (each dma_start costs ~0.6us of serial issue) and
ordered consts -> a -> b so the theta conv starts while b still streams.

PRECISION = "f32" (default): all matmuls in true fp32 (PE LOW+HIGH two-pass
mode), output matches the jax reference to ~7e-7 relative.
PRECISION = "f32r": TF32-like single-pass PE mode, 4x faster matmuls,
~3e-4 relative error end-to-end (measured on HW).
"""

import numpy as np

B, C, Ci, H, W = 4, 256, 128, 64, 64
N_PIX = H * W            # 4096 pixels per sample
N_CORES = 8
HALF = N_PIX // 2        # 2048 output pixels per core
P = 128
CC = C // P              # 2 channel chunks
RB = 512                 # row block (max 4-byte moving free dim)
MCH = N_PIX // P         # 32 pixel chunks for the Mi contraction
BN_EPS = 1e-5

PRECISION = "f32"        # "f32r" | "f32"

_CACHE = {}


def _build(precision=PRECISION):
    import concourse.bacc as bacc
    import concourse.mybir as mybir
    import concourse.tile as tile
    from concourse.masks import make_identity

    f32 = mybir.dt.float32
    fmm = mybir.dt.float32r if precision == "f32r" else f32
    Act = mybir.ActivationFunctionType

    # Bacc (not raw Bass): compile() legalizes sync waits (TRN2 allows at
    # most one sem wait per instruction; excess waits split onto
    # InstEventSemaphore / moved to ldweights).
    nc = bacc.Bacc("TRN2", num_devices=N_CORES)

    # packed weights: [thetaT(2x128) | phiT(2x128) | gwT(2x128) | WT(256)]
    wpack_d = nc.dram_tensor("wpack", [P, 4 * C], fmm, kind="ExternalInput")
    # packed f32 per-partition vectors:
    # [tb | pb | gb | scale cc0 | scale cc1 | shift cc0 | shift cc1]
    vpack_d = nc.dram_tensor("vpack", [P, 7], f32, kind="ExternalInput")
    a_d = nc.dram_tensor("a_half", [CC, P, HALF], fmm, kind="ExternalInput")
    b_d = nc.dram_tensor("b_full", [CC, P, N_PIX], fmm, kind="ExternalInput")
    out_d = nc.dram_tensor("out", [CC, P, HALF], f32, kind="ExternalOutput")

    with tile.TileContext(nc) as tc:
        with (
            tc.tile_pool(name="const", bufs=1) as cpool,
            tc.tile_pool(name="big", bufs=1) as bpool,
            tc.tile_pool(name="work", bufs=3) as wpool,
            tc.tile_pool(name="ps", bufs=2, space="PSUM") as ppool,
        ):
            # ---- constants: 2 DMAs -----------------------------------------
            wpack_sb = cpool.tile([P, 4 * C], fmm)
            vpack_sb = cpool.tile([P, 7], f32)
            nc.sync.dma_start(out=wpack_sb[:], in_=wpack_d[:])
            nc.sync.dma_start(out=vpack_sb[:], in_=vpack_d[:])
            thetaT_sb = wpack_sb[:, 0:C].rearrange("p (c k) -> p c k", c=CC)
            phiT_sb = wpack_sb[:, C : 2 * C].rearrange("p (c k) -> p c k", c=CC)
            gwT_sb = wpack_sb[:, 2 * C : 3 * C].rearrange("p (c k) -> p c k", c=CC)
            WT_sb = wpack_sb[:, 3 * C : 4 * C]
            tb_sb, pb_sb, gb_sb = (vpack_sb[:, i : i + 1] for i in range(3))
            scale_sb = vpack_sb[:, 3:5]
            shift_sb = vpack_sb[:, 5:7]

            # gpsimd memset/affine_select reject f32r and the BIR verifier
            # wants f32r matmul operands produced as f32r: build the identity
            # in f32, then round it into the matmul dtype with a DVE copy
            ident_f32 = cpool.tile([P, P], f32)
            ident_sb = cpool.tile([P, P], fmm)
            make_identity(nc, ident_f32[:])
            nc.vector.tensor_copy(ident_sb[:], ident_f32[:])

            # ---- activation loads: a first (theta), then b halves ----------
            a_sb = bpool.tile([P, CC, HALF], fmm)
            b_sb = bpool.tile([P, CC, N_PIX], fmm)
            for hh in range(2):
                s = hh * (HALF // 2)
                for cc in range(CC):
                    nc.sync.dma_start(
                        out=a_sb[:, cc, s : s + HALF // 2],
                        in_=a_d[cc, :, s : s + HALF // 2],
                    )
            for hh in range(2):
                s = hh * (N_PIX // 2)
                for cc in range(CC):
                    nc.sync.dma_start(
                        out=b_sb[:, cc, s : s + N_PIX // 2],
                        in_=b_d[cc, :, s : s + N_PIX // 2],
                    )

            # ---- theta conv: theta_x[Ci, HALF] (1/N + bias folded in) ------
            theta_x = bpool.tile([Ci, HALF], fmm)
            for p in range(HALF // RB):
                th_ps = ppool.tile([P, RB], f32, tag="conv", bufs=2, name="th_ps")
                for cc in range(CC):
                    nc.tensor.matmul(
                        th_ps[:],
                        thetaT_sb[:, cc, :],
                        a_sb[:, cc, p * RB : (p + 1) * RB],
                        start=(cc == 0),
                        stop=(cc == CC - 1),
                    )
                nc.scalar.activation(
                    theta_x[:, p * RB : (p + 1) * RB], th_ps[:], Act.Identity,
                    bias=tb_sb,
                )

            # ---- phi/g convs + transpose + Mi accumulation, per quarter ----
            phi_x = bpool.tile([Ci, N_PIX], fmm)
            g_x = bpool.tile([Ci, N_PIX], fmm)
            mi_ps = ppool.tile([Ci, Ci], f32, tag="mi", bufs=1, name="mi_ps")
            for q in range(4):
                for p in (2 * q, 2 * q + 1):
                    sl = slice(p * RB, (p + 1) * RB)
                    ph_ps = ppool.tile([P, RB], f32, tag="conv", bufs=2, name="ph_ps")
                    for cc in range(CC):
                        nc.tensor.matmul(
                            ph_ps[:],
                            phiT_sb[:, cc, :],
                            b_sb[:, cc, sl],
                            start=(cc == 0),
                            stop=(cc == CC - 1),
                        )
                    nc.scalar.activation(
                        phi_x[:, sl], ph_ps[:], Act.Identity, bias=pb_sb,
                    )
                    g_ps = ppool.tile([P, RB], f32, tag="conv", bufs=2, name="g_ps")
                    for cc in range(CC):
                        nc.tensor.matmul(
                            g_ps[:],
                            gwT_sb[:, cc, :],
                            b_sb[:, cc, sl],
                            start=(cc == 0),
                            stop=(cc == CC - 1),
                        )
                    nc.scalar.activation(
                        g_x[:, sl], g_ps[:], Act.Identity, bias=gb_sb,
                    )
                for m in range(8 * q, 8 * q + 8):
                    sl = slice(m * P, (m + 1) * P)
                    tpp_ps = ppool.tile([P, P], fmm, tag="tp", bufs=3, name="tpp_ps")
                    nc.tensor.transpose(tpp_ps[:], phi_x[:, sl], ident_sb[:])
                    phiT_m = wpool.tile([P, Ci], fmm, tag="phiT_m", bufs=3,
                                        name="phiT_m")
                    nc.vector.tensor_copy(phiT_m[:], tpp_ps[:])
                    tpg_ps = ppool.tile([P, P], fmm, tag="tp", bufs=3, name="tpg_ps")
                    nc.tensor.transpose(tpg_ps[:], g_x[:, sl], ident_sb[:])
                    gT_m = wpool.tile([P, Ci], fmm, tag="gT_m", bufs=3, name="gT_m")
                    nc.scalar.activation(gT_m[:], tpg_ps[:], Act.Copy)
                    nc.tensor.matmul(
                        mi_ps[:], phiT_m[:], gT_m[:],
                        start=(m == 0), stop=(m == MCH - 1),
                    )
            mi_sb = wpool.tile([Ci, Ci], fmm, tag="mi_sb", bufs=1, name="mi_sb")
            nc.vector.tensor_copy(mi_sb[:], mi_ps[:])

            # ---- y^T = Mi^T-contract theta_x; W conv; BN; store ------------
            for r in range(HALF // RB):
                rows = slice(r * RB, (r + 1) * RB)
                yt_ps = ppool.tile([Ci, RB], f32, tag="yt", bufs=2, name="yt_ps")
                nc.tensor.matmul(
                    yt_ps[:], mi_sb[:], theta_x[:, rows], start=True, stop=True,
                )
                yT_sb = wpool.tile([Ci, RB], fmm, tag="ysb", bufs=2, name="yT_sb")
                nc.vector.tensor_copy(yT_sb[:], yt_ps[:])
                osb = wpool.tile([P, CC, RB], f32, tag="osb", bufs=2, name="osb")
                for cc in range(CC):
                    wy_ps = ppool.tile([P, RB], f32, tag="conv", bufs=2, name="wy_ps")
                    nc.tensor.matmul(
                        wy_ps[:],
                        WT_sb[:, cc * P : (cc + 1) * P],
                        yT_sb[:],
                        start=True,
                        stop=True,
                    )
                    nc.scalar.activation(
                        osb[:, cc, :], wy_ps[:], Act.Identity,
                        bias=shift_sb[:, cc : cc + 1],
                        scale=scale_sb[:, cc : cc + 1],
                    )
                nc.sync.dma_start(
                    out=out_d[:, :, rows].rearrange("c p r -> p c r"), in_=osb[:],
                )

    nc.compile()
    return nc


def _get_nc():
    if "nc" not in _CACHE:
        _CACHE["nc"] = _build()
    return _CACHE["nc"]


def _prep_in_maps(a, b, theta_w, theta_b, phi_w, phi_b, g_w, g_b, W_w,
                  bn_gamma, bn_beta, bn_mean, bn_var):
    f = np.float32
    a4 = np.ascontiguousarray(np.asarray(a, f).reshape(B, C, N_PIX))
    b4 = np.ascontiguousarray(np.asarray(b, f).reshape(B, C, N_PIX))

    inv_n = 1.0 / np.float64(N_PIX)
    thetaT = (np.asarray(theta_w, f).T * inv_n).astype(f)   # [C, Ci]
    phiT = np.asarray(phi_w, f).T                           # [C, Ci]
    gwT = np.asarray(g_w, f).T                              # [C, Ci]
    WT = np.asarray(W_w, f).T                               # [Ci, C]
    # wpack rows: partition p; cols: [thetaT cc0|cc1 | phiT cc0|cc1 |
    #                                 gwT cc0|cc1 | WT]
    wpack = np.empty((P, 4 * C), f)
    for i, wT in enumerate((thetaT, phiT, gwT)):
        for cc in range(CC):
            wpack[:, i * C + cc * Ci : i * C + (cc + 1) * Ci] = \
                wT[cc * P : (cc + 1) * P, :]
    wpack[:, 3 * C : 4 * C] = WT
    wpack = np.ascontiguousarray(wpack)

    scale = (np.asarray(bn_gamma, f) / np.sqrt(np.asarray(bn_var, f) + BN_EPS)).astype(f)
    shift = (np.asarray(bn_beta, f) - np.asarray(bn_mean, f) * scale).astype(f)
    vpack = np.stack([
        (np.asarray(theta_b, f) * inv_n).astype(f),
        np.asarray(phi_b, f),
        np.asarray(g_b, f),
        scale[:P], scale[P:],
        shift[:P], shift[P:],
    ], axis=1)
    vpack = np.ascontiguousarray(vpack)

    in_maps = []
    for core in range(N_CORES):
        s, h = divmod(core, 2)
        in_maps.append({
            "a_half": np.ascontiguousarray(
                a4[s][:, h * HALF : (h + 1) * HALF].reshape(CC, P, HALF)),
            "b_full": np.ascontiguousarray(b4[s].reshape(CC, P, N_PIX)),
            "wpack": wpack,
            "vpack": vpack,
        })
    return in_maps


def run(inputs: dict, trace: bool = False):
    from concourse.bass_utils import run_bass_kernel_spmd

    nc = _get_nc()
    in_maps = _prep_in_maps(**inputs)
    res = run_bass_kernel_spmd(nc, in_maps, list(range(N_CORES)), trace=trace)
    out = np.empty((B, C, N_PIX), np.float32)
    for core in range(N_CORES):
        s, h = divmod(core, 2)
        out[s][:, h * HALF : (h + 1) * HALF] = res.results[core]["out"].reshape(C, HALF)
    return out.reshape(B, C, H, W), res


def kernel(**inputs) -> np.ndarray:
    out, _ = run(inputs, trace=False)
    return out
